# revision 3
# baseline (speedup 1.0000x reference)
"""Trainium2 Bass kernel for DenseEquivariantMatrix.

Math:  out[b, fo, g] = sum_{fi,h} x[b, fi, h] * kernel[fo, fi, pt[h, g]] + bias[fo]

A B x K x N matmul (K = fi*h = 8192, N = fo*g = 8192) whose weight matrix is a
gather of 32x32 blocks from the kernel table.  Sharding: tensor-parallel over
the output n_symm dim (32 g's per core, 8 cores).

Per-core dataflow (all dtypes float32r = fp32 bits, FP22 multiply, fp32 accum):
  - indirect-DMA gather, one whole 4KB kernel-table block per partition:
    G[h_loc, (g, fi, fo)] = KT[pt[h, g]]; 32 gathers per h-half (one per g),
    offsets are raw pt values (coef = 1024 from the table AP shape).
  - matmul rhs is a strided 3D AP into G at fixed fi: [h x (g,16) x (fo,32)]
    = 512 columns; lhsT is an X^T chunk [h x b] (host-pretransposed layout).
  - K accumulated in PSUM over 32 fi-chunks per h-half; h-half 2 adds bias
    via a K=1 ones^T @ bias_row matmul and accumulates into DRAM with a
    SWDGE accum_op=add DMA.
"""

import os
import numpy as np

B = 2048
F_IN = 32
F_OUT = 32
H = 256  # n_symm (contraction copy)
G = 256  # n_symm (output copy)
N_CORES = 8
G_CORE = G // N_CORES  # 32
K = F_IN * H  # 8192
N_COLS = G_CORE * F_OUT  # 1024 per core, cols ordered (g_local, fo)
BLK = F_IN * F_OUT  # 1024 elements per kernel-table block

TRACE = bool(int(os.environ.get("KERNEL_TRACE", "0")))
LAST_RESULTS = None

_PROGRAM = None


def _build_program():
    import concourse.bacc as bacc
    import concourse.bass as bass
    import concourse.mybir as mybir
    import concourse.tile as tile

    f32 = mybir.dt.float32
    f32r = mybir.dt.float32r
    i32 = mybir.dt.int32

    nc = bacc.Bacc(
        "TRN2", target_bir_lowering=False, debug=False, num_devices=N_CORES
    )

    xt = nc.dram_tensor("xt", (K, B), f32r, kind="ExternalInput").ap()
    kt = nc.dram_tensor("kt", (H, BLK), f32r, kind="ExternalInput").ap()
    ptg = nc.dram_tensor("ptg", (H, G_CORE), i32, kind="ExternalInput").ap()
    biasrow = nc.dram_tensor("biasrow", (1, N_COLS), f32r, kind="ExternalInput").ap()
    onesrow = nc.dram_tensor("onesrow", (1, 128), f32r, kind="ExternalInput").ap()
    out = nc.dram_tensor("out", (B, N_COLS), f32, kind="ExternalOutput").ap()

    M_BLK = B // 128  # 16

    with tile.TileContext(nc) as tc:
        with (
            tc.tile_pool(name="const", bufs=1) as const_pool,
            tc.tile_pool(name="g", bufs=1) as g_pool,
            tc.tile_pool(name="x", bufs=2) as x_pool,
            tc.tile_pool(name="o", bufs=2) as o_pool,
            tc.tile_pool(name="psum", bufs=2, space="PSUM") as psum_pool,
        ):
            # pts[p, hc*32+g] = pt[hc*128+p, g]
            pts = const_pool.tile([128, 2 * G_CORE], i32, tag="pts")
            nc.sync.dma_start(
                pts[:].rearrange("p (hc g) -> p hc g", hc=2),
                ptg.rearrange("(hc p) g -> p hc g", p=128),
            )
            bias_t = const_pool.tile([1, N_COLS], f32r, tag="bias")
            nc.sync.dma_start(bias_t[:], biasrow[:])
            ones_t = const_pool.tile([1, 128], f32r, tag="ones")
            nc.sync.dma_start(ones_t[:], onesrow[:])

            xt_r = xt.rearrange("(fi hc p) b -> fi hc p b", hc=2, p=128)

            for hc in range(2):
                Gt = g_pool.tile([128, G_CORE * BLK], f32r, tag="G")
                for g in range(G_CORE):
                    nc.gpsimd.indirect_dma_start(
                        out=Gt[:, g * BLK : (g + 1) * BLK],
                        out_offset=None,
                        in_=kt[:],
                        in_offset=bass.IndirectOffsetOnAxis(
                            ap=pts[:, hc * G_CORE + g : hc * G_CORE + g + 1],
                            axis=0,
                        ),
                    )
                G4 = Gt[:].rearrange(
                    "p (g fi fo) -> p g fi fo", g=G_CORE, fi=F_IN
                )

                for m in range(M_BLK):
                    xsl = x_pool.tile([128, F_IN * 128], f32r, tag="x")
                    nc.sync.dma_start(
                        xsl[:].rearrange("p (fi j) -> p fi j", fi=F_IN),
                        xt_r[:, hc, :, m * 128 : (m + 1) * 128].rearrange(
                            "fi p j -> p fi j"
                        ),
                    )
                    ps = psum_pool.tile([128, N_COLS], f32, tag="ps")
                    for fi in range(F_IN):
                        lhsT = xsl[:, fi * 128 : (fi + 1) * 128]
                        last = hc == 0 and fi == F_IN - 1
                        nc.tensor.matmul(
                            ps[:, 0:512],
                            lhsT=lhsT,
                            rhs=G4[:, 0:16, fi, :],
                            start=(fi == 0),
                            stop=last,
                        )
                        nc.tensor.matmul(
                            ps[:, 512:1024],
                            lhsT=lhsT,
                            rhs=G4[:, 16:32, fi, :],
                            start=(fi == 0),
                            stop=last,
                        )
                    if hc == 1:
                        nc.tensor.matmul(
                            ps[:, 0:512],
                            lhsT=ones_t[:],
                            rhs=bias_t[:, 0:512],
                            start=False,
                            stop=True,
                        )
                        nc.tensor.matmul(
                            ps[:, 512:1024],
                            lhsT=ones_t[:],
                            rhs=bias_t[:, 512:1024],
                            start=False,
                            stop=True,
                        )
                    ot = o_pool.tile([128, N_COLS], f32, tag="o")
                    nc.vector.tensor_copy(ot[:], ps[:])
                    if hc == 0:
                        nc.gpsimd.dma_start(
                            out[m * 128 : (m + 1) * 128, :], ot[:]
                        )
                    else:
                        nc.gpsimd.dma_start(
                            out[m * 128 : (m + 1) * 128, :],
                            ot[:],
                            accum_op=mybir.AluOpType.add,
                        )

    nc.compile()
    return nc


def _get_program():
    global _PROGRAM
    if _PROGRAM is None:
        _PROGRAM = _build_program()
    return _PROGRAM


def kernel(x, kernel, bias, product_table):
    global LAST_RESULTS
    from concourse import bass_utils

    x = np.asarray(x, dtype=np.float32)
    kernel = np.asarray(kernel, dtype=np.float32)
    bias = np.asarray(bias, dtype=np.float32)
    product_table = np.asarray(product_table, dtype=np.int32)

    nc = _get_program()

    # X^T with k = fi*H + h on rows
    xt = np.ascontiguousarray(x.transpose(1, 2, 0)).reshape(K, B)
    # kernel table KT[k][fi][fo]
    kt = np.ascontiguousarray(kernel.transpose(2, 1, 0)).reshape(H, BLK)
    bias_row = np.ascontiguousarray(np.tile(bias, G_CORE)[None, :])
    ones_row = np.ones((1, 128), np.float32)

    in_maps = []
    for c in range(N_CORES):
        in_maps.append(
            {
                "xt": xt,
                "kt": kt,
                "ptg": np.ascontiguousarray(
                    product_table[:, c * G_CORE : (c + 1) * G_CORE]
                ),
                "biasrow": bias_row,
                "onesrow": ones_row,
            }
        )

    res = bass_utils.run_bass_kernel_spmd(
        nc,
        in_maps,
        core_ids=list(range(N_CORES)),
        trace=TRACE,
        trace_cores=[0] if TRACE else None,
        tmpdir=os.environ.get("KERNEL_TMPDIR") or None,
    )
    LAST_RESULTS = res

    # per-core cols are (g_local, fo); assemble to (B, F_OUT, G)
    parts = [
        res.results[c]["out"].reshape(B, G_CORE, F_OUT).transpose(0, 2, 1)
        for c in range(N_CORES)
    ]
    return np.ascontiguousarray(np.concatenate(parts, axis=2), dtype=np.float32)


# revision 8
# speedup vs baseline: 1.3289x; 1.3289x over previous
"""Trainium2 Bass kernel for DenseEquivariantMatrix.

Math:  out[b, fo, g] = sum_{fi,h} x[b, fi, h] * kernel[fo, fi, pt[h, g]] + bias[fo]

A B x K x N matmul (K = fi*h = 8192, N = fo*g = 8192) whose weight matrix is a
gather of 32x32 blocks from the kernel table.  Sharding: tensor-parallel over
the output n_symm dim (32 g's per core, 8 cores).

Per-core dataflow (all dtypes float32r = fp32 bits, FP22 multiply, fp32 accum):
  - indirect-DMA gather, one whole 4KB kernel-table block per partition:
    G[h_loc, (g, fi, fo)] = KT[pt[h, g]]; 32 gathers per h-half (one per g),
    offsets are raw pt values (coef = 1024 from the table AP shape).
  - matmul rhs is a strided 3D AP into G at fixed fi: [h x (g,16) x (fo,32)]
    = 512 columns; lhsT is an X^T chunk [h x b] (host-pretransposed layout).
  - K accumulated in PSUM over 32 fi-chunks per h-half; h-half 2 adds bias
    via a K=1 ones^T @ bias_row matmul and accumulates into DRAM with a
    SWDGE accum_op=add DMA.
"""

import os
import numpy as np

B = 2048
F_IN = 32
F_OUT = 32
H = 256  # n_symm (contraction copy)
G = 256  # n_symm (output copy)
N_CORES = 8
G_CORE = G // N_CORES  # 32
K = F_IN * H  # 8192
N_COLS = G_CORE * F_OUT  # 1024 per core, cols ordered (g_local, fo)
BLK = F_IN * F_OUT  # 1024 elements per kernel-table block

TRACE = bool(int(os.environ.get("KERNEL_TRACE", "0")))
LAST_RESULTS = None

_PROGRAM = None


def _build_program():
    import concourse.bacc as bacc
    import concourse.bass as bass
    import concourse.mybir as mybir
    import concourse.tile as tile

    f32 = mybir.dt.float32
    f32r = mybir.dt.float32r
    i32 = mybir.dt.int32

    nc = bacc.Bacc(
        "TRN2", target_bir_lowering=False, debug=False, num_devices=N_CORES
    )

    # host-tiled X^T: xt[hc, m, p, fi, j] = x[m*128+j, fi, hc*128+p]
    # -> per (hc, m) slab, each partition p reads 16KB contiguous
    xt = nc.dram_tensor(
        "xt", (2, B // 128, 128, F_IN, 128), f32r, kind="ExternalInput"
    ).ap()
    kt = nc.dram_tensor("kt", (H, BLK), f32r, kind="ExternalInput").ap()
    ptg = nc.dram_tensor("ptg", (H, G_CORE), i32, kind="ExternalInput").ap()
    biasrow = nc.dram_tensor("biasrow", (1, N_COLS), f32r, kind="ExternalInput").ap()
    onesrow = nc.dram_tensor("onesrow", (1, 128), f32r, kind="ExternalInput").ap()
    out = nc.dram_tensor("out", (B, N_COLS), f32, kind="ExternalOutput").ap()

    M_BLK = B // 128  # 16

    with tile.TileContext(nc) as tc:
        with (
            tc.tile_pool(name="const", bufs=1) as const_pool,
            tc.tile_pool(name="g", bufs=1) as g_pool,
            tc.tile_pool(name="x", bufs=2) as x_pool,
            tc.tile_pool(name="o", bufs=2) as o_pool,
            tc.tile_pool(name="psum", bufs=2, space="PSUM") as psum_pool,
        ):
            # pts[p, hc*32+g] = pt[hc*128+p, g]
            pts = const_pool.tile([128, 2 * G_CORE], i32, tag="pts")
            nc.sync.dma_start(
                pts[:].rearrange("p (hc g) -> p hc g", hc=2),
                ptg.rearrange("(hc p) g -> p hc g", p=128),
            )
            bias_t = const_pool.tile([1, N_COLS], f32r, tag="bias")
            nc.sync.dma_start(bias_t[:], biasrow[:])
            ones_t = const_pool.tile([1, 128], f32r, tag="ones")
            nc.sync.dma_start(ones_t[:], onesrow[:])

            for hc in range(2):
                Gt = g_pool.tile([128, G_CORE * BLK], f32r, tag="G")
                for g in range(G_CORE):
                    nc.gpsimd.indirect_dma_start(
                        out=Gt[:, g * BLK : (g + 1) * BLK],
                        out_offset=None,
                        in_=kt[:],
                        in_offset=bass.IndirectOffsetOnAxis(
                            ap=pts[:, hc * G_CORE + g : hc * G_CORE + g + 1],
                            axis=0,
                        ),
                    )
                G4 = Gt[:].rearrange(
                    "p (g fi fo) -> p g fi fo", g=G_CORE, fi=F_IN
                )

                for m in range(M_BLK):
                    xsl = x_pool.tile([128, F_IN * 128], f32r, tag="x")
                    nc.sync.dma_start(
                        xsl[:],
                        xt[hc, m].rearrange("p fi j -> p (fi j)"),
                    )
                    ps = psum_pool.tile([128, N_COLS], f32, tag="ps")
                    for fi in range(F_IN):
                        lhsT = xsl[:, fi * 128 : (fi + 1) * 128]
                        last = hc == 0 and fi == F_IN - 1
                        nc.tensor.matmul(
                            ps[:, 0:512],
                            lhsT=lhsT,
                            rhs=G4[:, 0:16, fi, :],
                            start=(fi == 0),
                            stop=last,
                        )
                        nc.tensor.matmul(
                            ps[:, 512:1024],
                            lhsT=lhsT,
                            rhs=G4[:, 16:32, fi, :],
                            start=(fi == 0),
                            stop=last,
                        )
                    if hc == 1:
                        nc.tensor.matmul(
                            ps[:, 0:512],
                            lhsT=ones_t[:],
                            rhs=bias_t[:, 0:512],
                            start=False,
                            stop=True,
                        )
                        nc.tensor.matmul(
                            ps[:, 512:1024],
                            lhsT=ones_t[:],
                            rhs=bias_t[:, 512:1024],
                            start=False,
                            stop=True,
                        )
                    ot = o_pool.tile([128, N_COLS], f32, tag="o")
                    nc.vector.tensor_copy(ot[:], ps[:])
                    if hc == 0:
                        nc.sync.dma_start(
                            out[m * 128 : (m + 1) * 128, :], ot[:]
                        )
                    else:
                        nc.gpsimd.dma_start(
                            out[m * 128 : (m + 1) * 128, :],
                            ot[:],
                            accum_op=mybir.AluOpType.add,
                        )

    nc.compile()
    return nc


def _get_program():
    global _PROGRAM
    if _PROGRAM is None:
        _PROGRAM = _build_program()
    return _PROGRAM


def kernel(x, kernel, bias, product_table):
    global LAST_RESULTS
    from concourse import bass_utils

    x = np.asarray(x, dtype=np.float32)
    kernel = np.asarray(kernel, dtype=np.float32)
    bias = np.asarray(bias, dtype=np.float32)
    product_table = np.asarray(product_table, dtype=np.int32)

    nc = _get_program()

    # host-tiled X^T: xt[hc, m, p, fi, j] = x[m*128+j, fi, hc*128+p]
    xt = np.ascontiguousarray(
        x.reshape(B // 128, 128, F_IN, 2, 128).transpose(3, 0, 4, 2, 1)
    )
    # kernel table KT[k][fi][fo]
    kt = np.ascontiguousarray(kernel.transpose(2, 1, 0)).reshape(H, BLK)
    bias_row = np.ascontiguousarray(np.tile(bias, G_CORE)[None, :])
    ones_row = np.ones((1, 128), np.float32)

    in_maps = []
    for c in range(N_CORES):
        in_maps.append(
            {
                "xt": xt,
                "kt": kt,
                "ptg": np.ascontiguousarray(
                    product_table[:, c * G_CORE : (c + 1) * G_CORE]
                ),
                "biasrow": bias_row,
                "onesrow": ones_row,
            }
        )

    res = bass_utils.run_bass_kernel_spmd(
        nc,
        in_maps,
        core_ids=list(range(N_CORES)),
        trace=TRACE,
        trace_cores=[0] if TRACE else None,
        tmpdir=os.environ.get("KERNEL_TMPDIR") or None,
    )
    LAST_RESULTS = res

    # per-core cols are (g_local, fo); assemble to (B, F_OUT, G)
    parts = [
        res.results[c]["out"].reshape(B, G_CORE, F_OUT).transpose(0, 2, 1)
        for c in range(N_CORES)
    ]
    return np.ascontiguousarray(np.concatenate(parts, axis=2), dtype=np.float32)


# revision 11
# speedup vs baseline: 1.3792x; 1.0379x over previous
"""Trainium2 Bass kernel for DenseEquivariantMatrix.

Math:  out[b, fo, g] = sum_{fi,h} x[b, fi, h] * kernel[fo, fi, pt[h, g]] + bias[fo]

A B x K x N matmul (K = fi*h = 8192, N = fo*g = 8192) whose weight matrix is a
gather of 32x32 blocks from the kernel table.  Sharding: tensor-parallel over
the output n_symm dim (32 g's per core, 8 cores).

Per-core dataflow (all dtypes float32r = fp32 bits, FP22 multiply, fp32 accum):
  - indirect-DMA gather, one whole 4KB kernel-table block per partition:
    G[h_loc, (g, fi, fo)] = KT[pt[h, g]]; 32 gathers per h-half (one per g),
    offsets are raw pt values (coef = 1024 from the table AP shape).
  - matmul rhs is a strided 3D AP into G at fixed fi: [h x (g,16) x (fo,32)]
    = 512 columns; lhsT is an X^T chunk [h x b] (host-pretransposed layout).
  - K accumulated in PSUM over 32 fi-chunks per h-half; h-half 2 adds bias
    via a K=1 ones^T @ bias_row matmul and accumulates into DRAM with a
    SWDGE accum_op=add DMA.
"""

import os
import numpy as np

B = 2048
F_IN = 32
F_OUT = 32
H = 256  # n_symm (contraction copy)
G = 256  # n_symm (output copy)
N_CORES = 8
G_CORE = G // N_CORES  # 32
K = F_IN * H  # 8192
N_COLS = G_CORE * F_OUT  # 1024 per core, cols ordered (g_local, fo)
BLK = F_IN * F_OUT  # 1024 elements per kernel-table block

TRACE = bool(int(os.environ.get("KERNEL_TRACE", "0")))
LAST_RESULTS = None

_PROGRAM = None


def _build_program():
    import concourse.bacc as bacc
    import concourse.bass as bass
    import concourse.mybir as mybir
    import concourse.tile as tile

    f32 = mybir.dt.float32
    f32r = mybir.dt.float32r
    i32 = mybir.dt.int32

    nc = bacc.Bacc(
        "TRN2", target_bir_lowering=False, debug=False, num_devices=N_CORES
    )

    # host-tiled X^T: xt[hc, m, p, fi, j] = x[m*128+j, fi, hc*128+p]
    # -> per (hc, m) slab, each partition p reads 16KB contiguous
    xt = nc.dram_tensor(
        "xt", (2, B // 128, 128, F_IN, 128), f32r, kind="ExternalInput"
    ).ap()
    kt = nc.dram_tensor("kt", (H, BLK), f32r, kind="ExternalInput").ap()
    ptg = nc.dram_tensor("ptg", (H, G_CORE), i32, kind="ExternalInput").ap()
    biasrow = nc.dram_tensor("biasrow", (1, N_COLS), f32r, kind="ExternalInput").ap()
    onesrow = nc.dram_tensor("onesrow", (1, 128), f32r, kind="ExternalInput").ap()
    out = nc.dram_tensor("out", (B, N_COLS), f32, kind="ExternalOutput").ap()

    M_BLK = B // 128  # 16

    with tile.TileContext(nc) as tc:
        with (
            tc.tile_pool(name="const", bufs=1) as const_pool,
            tc.tile_pool(name="g", bufs=2) as g_pool,
            tc.tile_pool(name="x", bufs=2) as x_pool,
            tc.tile_pool(name="o", bufs=2) as o_pool,
            tc.tile_pool(name="psum", bufs=2, space="PSUM") as psum_pool,
        ):
            # pts[p, hc*32+g] = pt[hc*128+p, g]
            pts = const_pool.tile([128, 2 * G_CORE], i32, tag="pts")
            nc.sync.dma_start(
                pts[:].rearrange("p (hc g) -> p hc g", hc=2),
                ptg.rearrange("(hc p) g -> p hc g", p=128),
            )
            bias_t = const_pool.tile([1, N_COLS], f32r, tag="bias")
            nc.sync.dma_start(bias_t[:], biasrow[:])
            ones_t = const_pool.tile([1, 128], f32r, tag="ones")
            nc.sync.dma_start(ones_t[:], onesrow[:])

            NH = G_CORE // 2  # 16 g's per n-half panel
            for hc in range(2):
                G4s = []
                for nh in range(2):
                    Gt = g_pool.tile([128, NH * BLK], f32r, tag="G")
                    for g in range(NH):
                        gg = hc * G_CORE + nh * NH + g
                        nc.gpsimd.indirect_dma_start(
                            out=Gt[:, g * BLK : (g + 1) * BLK],
                            out_offset=None,
                            in_=kt[:],
                            in_offset=bass.IndirectOffsetOnAxis(
                                ap=pts[:, gg : gg + 1], axis=0
                            ),
                        )
                    G4s.append(
                        Gt[:].rearrange("p (g fi fo) -> p g fi fo", g=NH, fi=F_IN)
                    )

                for m in range(M_BLK):
                    xsl = x_pool.tile([128, F_IN * 128], f32r, tag="x")
                    nc.sync.dma_start(
                        xsl[:],
                        xt[hc, m].rearrange("p fi j -> p (fi j)"),
                    )
                    ps = psum_pool.tile([128, N_COLS], f32, tag="ps")
                    for fi in range(F_IN):
                        lhsT = xsl[:, fi * 128 : (fi + 1) * 128]
                        last = hc == 0 and fi == F_IN - 1
                        nc.tensor.matmul(
                            ps[:, 0:512],
                            lhsT=lhsT,
                            rhs=G4s[0][:, :, fi, :],
                            start=(fi == 0),
                            stop=last,
                        )
                        nc.tensor.matmul(
                            ps[:, 512:1024],
                            lhsT=lhsT,
                            rhs=G4s[1][:, :, fi, :],
                            start=(fi == 0),
                            stop=last,
                        )
                    if hc == 1:
                        nc.tensor.matmul(
                            ps[:, 0:512],
                            lhsT=ones_t[:],
                            rhs=bias_t[:, 0:512],
                            start=False,
                            stop=True,
                        )
                        nc.tensor.matmul(
                            ps[:, 512:1024],
                            lhsT=ones_t[:],
                            rhs=bias_t[:, 512:1024],
                            start=False,
                            stop=True,
                        )
                    ot = o_pool.tile([128, N_COLS], f32, tag="o")
                    nc.vector.tensor_copy(ot[:], ps[:])
                    if hc == 0:
                        nc.sync.dma_start(
                            out[m * 128 : (m + 1) * 128, :], ot[:]
                        )
                    else:
                        nc.gpsimd.dma_start(
                            out[m * 128 : (m + 1) * 128, :],
                            ot[:],
                            accum_op=mybir.AluOpType.add,
                        )

    nc.compile()
    return nc


def _get_program():
    global _PROGRAM
    if _PROGRAM is None:
        _PROGRAM = _build_program()
    return _PROGRAM


def kernel(x, kernel, bias, product_table):
    global LAST_RESULTS
    from concourse import bass_utils

    x = np.asarray(x, dtype=np.float32)
    kernel = np.asarray(kernel, dtype=np.float32)
    bias = np.asarray(bias, dtype=np.float32)
    product_table = np.asarray(product_table, dtype=np.int32)

    nc = _get_program()

    # host-tiled X^T: xt[hc, m, p, fi, j] = x[m*128+j, fi, hc*128+p]
    xt = np.ascontiguousarray(
        x.reshape(B // 128, 128, F_IN, 2, 128).transpose(3, 0, 4, 2, 1)
    )
    # kernel table KT[k][fi][fo]
    kt = np.ascontiguousarray(kernel.transpose(2, 1, 0)).reshape(H, BLK)
    bias_row = np.ascontiguousarray(np.tile(bias, G_CORE)[None, :])
    ones_row = np.ones((1, 128), np.float32)

    in_maps = []
    for c in range(N_CORES):
        in_maps.append(
            {
                "xt": xt,
                "kt": kt,
                "ptg": np.ascontiguousarray(
                    product_table[:, c * G_CORE : (c + 1) * G_CORE]
                ),
                "biasrow": bias_row,
                "onesrow": ones_row,
            }
        )

    res = bass_utils.run_bass_kernel_spmd(
        nc,
        in_maps,
        core_ids=list(range(N_CORES)),
        trace=TRACE,
        trace_cores=[0] if TRACE else None,
        tmpdir=os.environ.get("KERNEL_TMPDIR") or None,
    )
    LAST_RESULTS = res

    # per-core cols are (g_local, fo); assemble to (B, F_OUT, G)
    parts = [
        res.results[c]["out"].reshape(B, G_CORE, F_OUT).transpose(0, 2, 1)
        for c in range(N_CORES)
    ]
    return np.ascontiguousarray(np.concatenate(parts, axis=2), dtype=np.float32)
